# revision 22
# baseline (speedup 1.0000x reference)
"""Trainium2 Bass kernel for nn_Box_Rel_Classifier (v3).

Math (i over box2 rows, j over box1 rows, d over dims):
  z  = sigmoid(x0 - softplus(10*x1)/10),  Z = sigmoid(x0 + softplus(10*x1)/10)
  out_min[i*160+j, d] ~= max(z2[i,d], z1[j,d])   (gumbel log1p term dropped,
  out_max[i*160+j, d] ~= min(Z2[i,d], Z1[j,d])    abs err <= gb*ln2 ~ 2.5e-3)

All table math is on the 255*sigmoid scale; outputs u8 (host /255).
sigmoid is computed as exp(-ln(1+exp(+/-v))) so the whole prep uses one
ACT table set (Abs/Exp/Ln) -- no table reloads -- and the final Exp
emits fp16(255*z) directly via bias=ln(255).

Per-core schedule: box2 sharded 8 ways (128 rows = partition dim). Per
tensor 20 units of 2048 cols (8 j x 256 d) in 10 groups of 4096. The
max runs on DVE as fp16 tensor_tensor; broadcast operand per group from
one of 4 paths (pattern-tunable via KERNEL_PAT0/1):
  a: PE ones-matmul -> PSUM, ACT copy -> fp16
  b: GPSIMD partition_broadcast of the fp16 table row (t0 only)
  c: HWDGE stride-0 broadcast read of the DRAM fp16 table
  r: PE -> PSUM, DVE STT -> u8, HWDGE store (relieves the cast queue)
fp16 outputs leave via the gpsimd SWDGE casting DMA (fp16->u8).
Producers are emitted LAG groups ahead of the DVE consumer ops so the
DVE queue (the bottleneck, ~58us) never head-of-line blocks.
"""

import os
import sys

import numpy as np

try:
    import concourse.bacc as bacc  # noqa: F401
except ImportError:
    for p in ("/root/.axon_site/_ro/trn_rl_repo", "/opt/trn_rl_repo"):
        if p not in sys.path:
            sys.path.insert(0, p)
    import concourse.bacc as bacc

import concourse.bacc as bacc
import concourse.hw_specs as hw_specs
import concourse.tile as tile
from concourse import mybir
from concourse.bass_utils import run_bass_kernel_spmd

AF = mybir.ActivationFunctionType

# ---- activation-table patch: every ACT func this kernel uses (Abs, Exp,
# Ln, Copy) lives ONLY in natural_log_exp_and_others, so exactly one
# ACT_TABLE_LOAD is emitted (the scheduler freely interleaves prep chains
# with main-loop copies and would otherwise thrash table sets).
_orig_gat = hw_specs.get_activation_tables


def _patched_gat(arch):
    tabs = _orig_gat(arch)
    mine = {AF.Abs, AF.Exp, AF.Ln, AF.Copy, AF.Sigmoid}
    out = {}
    for name, funcs in tabs.items():
        if name == "natural_log_exp_and_others":
            out[name] = funcs | {AF.Copy}
        else:
            out[name] = funcs - mine
    return out


bacc.get_activation_tables = _patched_gat
ALU = mybir.AluOpType
F32 = mybir.dt.float32
F16 = mybir.dt.float16
U8 = mybir.dt.uint8

N1, N2, D = 160, 1024, 256
NCORES = 8
SH = N2 // NCORES          # 128 box2 rows per core
FLAT = N1 * D              # 40960 free cols per tensor
UNIT = 2048                # 8 j-rows
GW = 2                     # units per group
GCOL = GW * UNIT           # 4096
GJ = GW * 8                # 16 j-rows per group
NG = FLAT // GCOL          # 10 groups per tensor
LN255 = float(np.log(255.0))

# Per-tensor path pattern over the 10 groups.  b only valid for t0.
PAT0 = os.environ.get("KERNEL_PAT0", "cacaacaarr")
PAT1 = os.environ.get("KERNEL_PAT1", "acacaacaar")
# r-groups must be a contiguous tail per tensor (u8 output tensor range)
R_SPLIT = [8, 9]   # first r-group index per tensor
LAG = int(os.environ.get("KERNEL_LAG", "5"))

_CACHE = {}


def _build():
    nc = bacc.Bacc("TRN2", target_bir_lowering=False, debug=False)

    box1 = nc.dram_tensor("box1s", [N1, 2, D], F32, kind="ExternalInput").ap()
    box2 = nc.dram_tensor("box2s", [SH, 2, D], F32, kind="ExternalInput").ap()
    selw_in = nc.dram_tensor("selw", [8, 8 * 128], F16,
                             kind="ExternalInput").ap()
    outs = [
        nc.dram_tensor("out_min", [SH, R_SPLIT[0] * GJ, D], F16,
                       kind="ExternalOutput").ap(),
        nc.dram_tensor("out_max", [SH, R_SPLIT[1] * GJ, D], F16,
                       kind="ExternalOutput").ap(),
    ]
    outs8 = [
        nc.dram_tensor("out_min8", [SH, N1 - R_SPLIT[0] * GJ, D], U8,
                       kind="ExternalOutput").ap(),
        nc.dram_tensor("out_max8", [SH, N1 - R_SPLIT[1] * GJ, D], U8,
                       kind="ExternalOutput").ap(),
    ]
    rings = [nc.scalar, nc.sync]   # per-tensor HWDGE ring

    with tile.TileContext(nc) as tc:
        with (
            tc.tile_pool(name="persist", bufs=1) as persist,
            tc.tile_pool(name="dram", bufs=1, space="DRAM") as dram,
            tc.tile_pool(name="zhp", bufs=2) as zhp,
            tc.tile_pool(name="zbcp", bufs=4) as zbcp,
            tc.tile_pool(name="osbp", bufs=2) as osbp,
            tc.tile_pool(name="osb8p", bufs=1) as osb8p,
            tc.tile_pool(name="psum", bufs=2, space="PSUM") as psum,
        ):
            # fp16 tables: rows 0/64 = t0 (dup), rows 32/96 = t1 (dup);
            # duplicates let consecutive matmuls alternate PE tile positions
            # so LDWEIGHTS overlaps the previous matmul.
            tab16 = persist.tile([97, FLAT], F16)
            w16 = persist.tile([97, 128], F16)
            nc.vector.memset(w16[:], 1.0)
            rep4k = [persist.tile([128, GCOL], F16, tag=f"rep{t}",
                                  name=f"rep{t}") for t in range(2)]
            ln255 = persist.tile([128, 1], F32, tag="ln255")
            nc.vector.memset(ln255[:], LN255)
            # e-path: selector matmuls read the j-pair-rows table (no DRAM
            # row roundtrip) for group 0 of each tensor during the ramp.
            selw = persist.tile([8, 8 * 128], F16, tag="selw")
            nc.sync.dma_start(out=selw[:], in_=selw_in[:, :])
            rsh = [persist.tile([8, 512], F16, tag=f"rsh{t}",
                                name=f"rsh{t}") for t in range(2)]
            zscr = dram.tile([2, FLAT], F16)

            with tc.tile_pool(name="prep", bufs=1) as prep:
                # inputs
                xa = prep.tile([128, 2 * D], F32, tag="xa")
                nc.scalar.dma_start(
                    out=xa[:], in_=box1[0:128].rearrange("j c d -> j (c d)"))
                x0_a, x1_a = xa[:, 0:D], xa[:, D:2 * D]
                x2 = prep.tile([SH, 2 * D], F32, tag="x2")
                nc.sync.dma_start(
                    out=x2[:], in_=box2[:].rearrange("i c d -> i (c d)"))
                x0_2, x1_2 = x2[:, 0:D], x2[:, D:2 * D]
                xb = prep.tile([32, 2 * D], F32, tag="xa")
                nc.scalar.dma_start(
                    out=xb[:],
                    in_=box1[128:160].rearrange("j c d -> j (c d)"))
                x0_b, x1_b = xb[:, 0:D], xb[:, D:2 * D]

                def chain(xs, p, nm):
                    """hh = fp16(255*[zmin | zmax]) per (x0, x1) in xs,
                    paired into one [p, len(xs)*2*D] ACT stream.

                    half0: v = sp - x0, zmin = sigmoid(-v) = exp(-ln(1+e^v))
                    half1: w = -(sp + x0), zmax = sigmoid(+sp+x0) =
                           exp(-ln(1+e^w));  sp = 0.1*ln(1+exp(-10|x1|))
                           + max(x1, 0).
                    """
                    nx = len(xs)
                    u1 = prep.tile([p, nx * D], F32, tag="u1",
                                   name=f"u1{nm}")
                    for k, (x0, x1) in enumerate(xs):
                        nc.vector.scalar_tensor_tensor(
                            out=u1[:, k * D:(k + 1) * D], in0=x1[:],
                            scalar=-1.0, in1=x1[:], op0=ALU.mult,
                            op1=ALU.max)
                    e1 = prep.tile([p, nx * D], F32, tag="e1",
                                   name=f"e1{nm}")
                    nc.scalar.activation(e1[:], u1[:], AF.Exp, scale=-10.0)
                    l1 = prep.tile([p, nx * D], F32, tag=f"l1{nm}",
                                   name=f"l1{nm}")
                    nc.scalar.activation(l1[:], e1[:], AF.Ln, bias=1.0)
                    vv = prep.tile([p, nx * 2 * D], F32, tag="vv",
                                   name=f"vv{nm}")
                    for k, (x0, x1) in enumerate(xs):
                        l1k = l1[:, k * D:(k + 1) * D]
                        q0 = prep.tile([p, D], F32, tag="q0",
                                       name=f"q0{nm}{k}")
                        nc.vector.scalar_tensor_tensor(
                            out=q0[:], in0=x1[:], scalar=0.0, in1=x0[:],
                            op0=ALU.max, op1=ALU.subtract)
                        nc.vector.scalar_tensor_tensor(
                            out=vv[:, (2 * k) * D:(2 * k + 1) * D],
                            in0=l1k, scalar=0.1, in1=q0[:],
                            op0=ALU.mult, op1=ALU.add)
                        q1 = prep.tile([p, D], F32, tag="q0",
                                       name=f"q1{nm}{k}")
                        nc.vector.scalar_tensor_tensor(
                            out=q1[:], in0=x1[:], scalar=0.0, in1=x0[:],
                            op0=ALU.max, op1=ALU.add)
                        nc.vector.scalar_tensor_tensor(
                            out=vv[:, (2 * k + 1) * D:(2 * k + 2) * D],
                            in0=l1k, scalar=-0.1, in1=q1[:],
                            op0=ALU.mult, op1=ALU.subtract)
                    ee = prep.tile([p, nx * 2 * D], F32, tag="u1",
                                   name=f"ee{nm}")
                    nc.scalar.activation(ee[:], vv[:], AF.Exp)
                    ll = prep.tile([p, nx * 2 * D], F32, tag="vv",
                                   name=f"ll{nm}")
                    nc.scalar.activation(ll[:], ee[:], AF.Ln, bias=1.0)
                    hh = prep.tile([p, nx * 2 * D], F16, tag="hh",
                                   name=f"hh{nm}")
                    nc.scalar.activation(hh[:], ll[:], AF.Exp, scale=-1.0,
                                         bias=ln255[0:p, :])
                    return [[hh[:, (2 * k) * D:(2 * k + 1) * D],
                             hh[:, (2 * k + 1) * D:(2 * k + 2) * D]]
                            for k in range(nx)]

                # box2 + box1a paired (both 128 partitions)
                h2, ha = chain([(x0_2, x1_2), (x0_a, x1_a)], 128, "2a")
                for t in range(2):
                    nc.vector.tensor_copy(
                        out=rep4k[t][:, 0:UNIT]
                        .rearrange("p (r d) -> p r d", d=D),
                        in_=h2[t][:, None, :]
                        .broadcast_to([SH, UNIT // D, D]))
                    nc.vector.tensor_copy(out=rep4k[t][:, UNIT:GCOL],
                                          in_=rep4k[t][:, 0:UNIT])
                for t in range(2):
                    ring = rings[t]
                    ring.dma_start(
                        out=zscr[t:t + 1, 0:128 * D]
                        .rearrange("t (r d) -> (t r) d", d=D),
                        in_=ha[t])
                    ring.dma_start(
                        out=rsh[t][:],
                        in_=zscr[t:t + 1, 0:8 * 512]
                        .rearrange("t (q n) -> (t q) n", n=512))
                    ring.dma_start(out=tab16[32 * t:32 * t + 1, 0:128 * D],
                                   in_=zscr[t:t + 1, 0:128 * D])
                    ring.dma_start(
                        out=tab16[64 + 32 * t:64 + 32 * t + 1, 0:128 * D],
                        in_=zscr[t:t + 1, 0:128 * D])

                # b-chunk tables
                (hb,) = chain([(x0_b, x1_b)], 32, "b")
                for t in range(2):
                    ring = rings[t]
                    ring.dma_start(
                        out=zscr[t:t + 1, 128 * D:FLAT]
                        .rearrange("t (r d) -> (t r) d", d=D),
                        in_=hb[t])
                    ring.dma_start(out=tab16[32 * t:32 * t + 1, 128 * D:FLAT],
                                   in_=zscr[t:t + 1, 128 * D:FLAT])
                    ring.dma_start(
                        out=tab16[64 + 32 * t:64 + 32 * t + 1, 128 * D:FLAT],
                        in_=zscr[t:t + 1, 128 * D:FLAT])

            # ---------------- main loop ----------------
            ops = [ALU.max, ALU.min]
            pats = [PAT0, PAT1]

            def mm(ps, t, off):
                for c in range(4):
                    row = 32 * t + 64 * (c % 2)
                    nc.tensor.matmul(
                        ps[:, c * 512:(c + 1) * 512],
                        lhsT=w16[row:row + 1, :],
                        rhs=tab16[row:row + 1,
                                  off + c * 512:off + c * 512 + 512],
                        start=True, stop=True, tile_position=(row, 0))

            def prod_a(t, g):
                zh = zhp.tile([128, GCOL], F16, tag="zh", name=f"zh{t}_{g}")
                for h in range(GW):
                    ps = psum.tile([128, UNIT], F32, tag="ps",
                                   name=f"ps{t}_{g}_{h}")
                    mm(ps, t, g * GCOL + h * UNIT)
                    nc.scalar.activation(zh[:, h * UNIT:(h + 1) * UNIT],
                                         ps[:], AF.Copy)
                return zh

            def prod_e(t, g):
                zh = zhp.tile([128, GCOL], F16, tag="zh", name=f"zhe{t}_{g}")
                for h in range(GW):
                    ps = psum.tile([128, UNIT], F32, tag="ps",
                                   name=f"pse{t}_{g}_{h}")
                    for c in range(4):
                        q = (g * GCOL + h * UNIT) // 512 + c
                        nc.tensor.matmul(
                            ps[:, c * 512:(c + 1) * 512],
                            lhsT=selw[:, q * 128:(q + 1) * 128],
                            rhs=rsh[t][:, 0:512],
                            start=True, stop=True, tile_position=(0, 0))
                    nc.scalar.activation(zh[:, h * UNIT:(h + 1) * UNIT],
                                         ps[:], AF.Copy)
                return zh

            def prod_s(t, g):
                zb = zbcp.tile([128, GCOL], F16, tag="zbc",
                               name=f"zbs{t}_{g}")
                nc.gpsimd.dma_start(
                    out=zb[:],
                    in_=zscr[t:t + 1, g * GCOL:(g + 1) * GCOL]
                    .broadcast_to([128, GCOL]))
                return zb

            c_ctr = [0]

            def prod_c(t, g):
                zb = zbcp.tile([128, GCOL], F16, tag="zbc",
                               name=f"zbc{t}_{g}")
                c_ctr[0] += 1
                rings[c_ctr[0] % 2].dma_start(
                    out=zb[:],
                    in_=zscr[t:t + 1, g * GCOL:(g + 1) * GCOL]
                    .broadcast_to([128, GCOL]))
                return zb

            def prod_r(t, g):
                pss = []
                for h in range(GW):
                    ps = psum.tile([128, UNIT], F32, tag="ps",
                                   name=f"psr{t}_{g}_{h}")
                    mm(ps, t, g * GCOL + h * UNIT)
                    pss.append(ps)
                return pss

            st_ctr = [0]
            osb_pairs = {}
            st_ctr = [0]

            def cons_f16(t, g, zb):
                pair = g // 2
                if (t, pair) not in osb_pairs:
                    osb_pairs[(t, pair)] = osbp.tile(
                        [128, 2 * GCOL], F16, tag="osb",
                        name=f"osb{t}_{pair}")
                osb = osb_pairs[(t, pair)]
                sl = g % 2
                nc.vector.tensor_tensor(
                    out=osb[:, sl * GCOL:(sl + 1) * GCOL], in0=zb[:],
                    in1=rep4k[t][:], op=ops[t])
                if sl == 1:
                    j0 = pair * 2 * GJ
                    st_ctr[0] += 1
                    rings[st_ctr[0] % 2].dma_start(
                        out=outs[t][:, j0:j0 + 2 * GJ, :],
                        in_=osb_pairs.pop((t, pair))[:]
                        .rearrange("p (r d) -> p r d", d=D))

            def cons_r(t, g, pss):
                osb8 = osb8p.tile([128, GCOL], U8, tag="osb8",
                                  name=f"osb8{t}_{g}")
                for h in range(GW):
                    nc.vector.scalar_tensor_tensor(
                        out=osb8[:, h * UNIT:(h + 1) * UNIT], in0=pss[h][:],
                        scalar=0.0, in1=rep4k[t][:, 0:UNIT],
                        op0=ALU.bypass, op1=ops[t])
                j0 = (g - R_SPLIT[t]) * GJ
                nc.sync.dma_start(
                    out=outs8[t][:, j0:j0 + GJ, :],
                    in_=osb8[:].rearrange("p (r d) -> p r d", d=D))

            producers = {"a": prod_a, "c": prod_c, "s": prod_s,
                         "e": prod_e, "r": prod_r}


            jobs = []
            for k in range(NG):
                for t, ordl in ((0, ORD0), (1, ORD1)):
                    g = ordl[k]
                    jobs.append((t, g, pats[t][g]))

            staged = {}
            n = len(jobs)
            for i in range(n + LAG):
                if i < n:
                    t, g, p = jobs[i]
                    staged[i] = producers[p](t, g)
                k = i - LAG
                if k >= 0:
                    t, g, p = jobs[k]
                    if p == "r":
                        cons_r(t, g, staged.pop(k))
                    else:
                        cons_f16(t, g, staged.pop(k))
            # flush leftover half-filled fp16 pairs
            for (t, pair), osb in list(osb_pairs.items()):
                j0 = pair * 2 * GJ
                st_ctr[0] += 1
                rings[st_ctr[0] % 2].dma_start(
                    out=outs[t][:, j0:j0 + GJ, :],
                    in_=osb[:, 0:GCOL].rearrange("p (r d) -> p r d", d=D))
                del osb_pairs[(t, pair)]

    nc.compile()
    return nc


def _get_nc():
    if "nc" not in _CACHE:
        _CACHE["nc"] = _build()
    return _CACHE["nc"]


_SELW = np.zeros([8, 8 * 128], dtype=np.float16)
for _q in range(8):
    _SELW[_q, _q * 128:(_q + 1) * 128] = 1.0


def make_in_maps(box1s, box2s):
    box1s = np.ascontiguousarray(np.asarray(box1s, dtype=np.float32))
    box2s = np.ascontiguousarray(np.asarray(box2s, dtype=np.float32))
    return [
        {
            "box1s": box1s,
            "box2s": np.ascontiguousarray(box2s[c * SH:(c + 1) * SH]),
            "selw": _SELW,
        }
        for c in range(NCORES)
    ]


def kernel(box1s, box2s):
    nc = _get_nc()
    res = run_bass_kernel_spmd(nc, make_in_maps(box1s, box2s),
                               core_ids=list(range(NCORES)))
    inv = np.float32(1.0) / np.float32(255.0)

    def full(name, name8):
        parts = []
        for r in res.results:
            f = r[name].astype(np.float32)
            u = r[name8].astype(np.float32)
            parts.append(np.concatenate([f, u], axis=1).reshape(SH * N1, D))
        return np.concatenate(parts, axis=0) * inv

    return full("out_min", "out_min8"), full("out_max", "out_max8")


# revision 23
# speedup vs baseline: 1.1196x; 1.1196x over previous
"""Trainium2 Bass kernel for nn_Box_Rel_Classifier (v3).

Math (i over box2 rows, j over box1 rows, d over dims):
  z  = sigmoid(x0 - softplus(10*x1)/10),  Z = sigmoid(x0 + softplus(10*x1)/10)
  out_min[i*160+j, d] ~= max(z2[i,d], z1[j,d])   (gumbel log1p term dropped,
  out_max[i*160+j, d] ~= min(Z2[i,d], Z1[j,d])    abs err <= gb*ln2 ~ 2.5e-3)

All table math is on the 255*sigmoid scale; outputs u8 (host /255).
sigmoid is computed as exp(-ln(1+exp(+/-v))) so the whole prep uses one
ACT table set (Abs/Exp/Ln) -- no table reloads -- and the final Exp
emits fp16(255*z) directly via bias=ln(255).

Per-core schedule: box2 sharded 8 ways (128 rows = partition dim). Per
tensor 20 units of 2048 cols (8 j x 256 d) in 10 groups of 4096. The
max runs on DVE as fp16 tensor_tensor; broadcast operand per group from
one of 4 paths (pattern-tunable via KERNEL_PAT0/1):
  a: PE ones-matmul -> PSUM, ACT copy -> fp16
  b: GPSIMD partition_broadcast of the fp16 table row (t0 only)
  c: HWDGE stride-0 broadcast read of the DRAM fp16 table
  r: PE -> PSUM, DVE STT -> u8, HWDGE store (relieves the cast queue)
fp16 outputs leave via the gpsimd SWDGE casting DMA (fp16->u8).
Producers are emitted LAG groups ahead of the DVE consumer ops so the
DVE queue (the bottleneck, ~58us) never head-of-line blocks.
"""

import os
import sys

import numpy as np

try:
    import concourse.bacc as bacc  # noqa: F401
except ImportError:
    for p in ("/root/.axon_site/_ro/trn_rl_repo", "/opt/trn_rl_repo"):
        if p not in sys.path:
            sys.path.insert(0, p)
    import concourse.bacc as bacc

import concourse.bacc as bacc
import concourse.hw_specs as hw_specs
import concourse.tile as tile
from concourse import mybir
from concourse.bass_utils import run_bass_kernel_spmd

AF = mybir.ActivationFunctionType

# ---- activation-table patch: every ACT func this kernel uses (Abs, Exp,
# Ln, Copy) lives ONLY in natural_log_exp_and_others, so exactly one
# ACT_TABLE_LOAD is emitted (the scheduler freely interleaves prep chains
# with main-loop copies and would otherwise thrash table sets).
_orig_gat = hw_specs.get_activation_tables


def _patched_gat(arch):
    tabs = _orig_gat(arch)
    mine = {AF.Abs, AF.Exp, AF.Ln, AF.Copy, AF.Sigmoid}
    out = {}
    for name, funcs in tabs.items():
        if name == "natural_log_exp_and_others":
            out[name] = funcs | {AF.Copy}
        else:
            out[name] = funcs - mine
    return out


bacc.get_activation_tables = _patched_gat
ALU = mybir.AluOpType
F32 = mybir.dt.float32
F16 = mybir.dt.float16
U8 = mybir.dt.uint8

N1, N2, D = 160, 1024, 256
NCORES = 8
SH = N2 // NCORES          # 128 box2 rows per core
FLAT = N1 * D              # 40960 free cols per tensor
UNIT = 2048                # 8 j-rows
GW = 2                     # units per group
GCOL = GW * UNIT           # 4096
GJ = GW * 8                # 16 j-rows per group
NG = FLAT // GCOL          # 10 groups per tensor
LN255 = float(np.log(255.0))

# Per-tensor path pattern over the 10 groups.  b only valid for t0.
PAT0 = os.environ.get("KERNEL_PAT0", "cacaacaarr")
PAT1 = os.environ.get("KERNEL_PAT1", "acacaacaar")
# r-groups must be a contiguous tail per tensor (u8 output tensor range)
R_SPLIT = [8, 9]   # first r-group index per tensor
LAG = int(os.environ.get("KERNEL_LAG", "5"))

_CACHE = {}


def _build():
    nc = bacc.Bacc("TRN2", target_bir_lowering=False, debug=False)

    box1 = nc.dram_tensor("box1s", [N1, 2, D], F32, kind="ExternalInput").ap()
    box2 = nc.dram_tensor("box2s", [SH, 2, D], F32, kind="ExternalInput").ap()
    outs = [
        nc.dram_tensor("out_min", [SH, R_SPLIT[0] * GJ, D], F16,
                       kind="ExternalOutput").ap(),
        nc.dram_tensor("out_max", [SH, R_SPLIT[1] * GJ, D], F16,
                       kind="ExternalOutput").ap(),
    ]
    outs8 = [
        nc.dram_tensor("out_min8", [SH, N1 - R_SPLIT[0] * GJ, D], U8,
                       kind="ExternalOutput").ap(),
        nc.dram_tensor("out_max8", [SH, N1 - R_SPLIT[1] * GJ, D], U8,
                       kind="ExternalOutput").ap(),
    ]
    rings = [nc.scalar, nc.sync]   # per-tensor HWDGE ring

    with tile.TileContext(nc) as tc:
        with (
            tc.tile_pool(name="persist", bufs=1) as persist,
            tc.tile_pool(name="dram", bufs=1, space="DRAM") as dram,
            tc.tile_pool(name="zhp", bufs=2) as zhp,
            tc.tile_pool(name="zbcp", bufs=4) as zbcp,
            tc.tile_pool(name="osbp", bufs=2) as osbp,
            tc.tile_pool(name="osb8p", bufs=1) as osb8p,
            tc.tile_pool(name="psum", bufs=2, space="PSUM") as psum,
        ):
            # fp16 tables: rows 0/64 = t0 (dup), rows 32/96 = t1 (dup);
            # duplicates let consecutive matmuls alternate PE tile positions
            # so LDWEIGHTS overlaps the previous matmul.
            tab16 = persist.tile([97, FLAT], F16)
            w16 = persist.tile([97, 128], F16)
            nc.vector.memset(w16[:], 1.0)
            rep4k = [persist.tile([128, GCOL], F16, tag=f"rep{t}",
                                  name=f"rep{t}") for t in range(2)]
            ln255 = persist.tile([128, 1], F32, tag="ln255")
            nc.vector.memset(ln255[:], LN255)
            zscr = dram.tile([2, FLAT], F16)

            with tc.tile_pool(name="prep", bufs=1) as prep:
                # inputs
                xa = prep.tile([128, 2 * D], F32, tag="xa")
                nc.scalar.dma_start(
                    out=xa[:], in_=box1[0:128].rearrange("j c d -> j (c d)"))
                x0_a, x1_a = xa[:, 0:D], xa[:, D:2 * D]
                x2 = prep.tile([SH, 2 * D], F32, tag="x2")
                nc.sync.dma_start(
                    out=x2[:], in_=box2[:].rearrange("i c d -> i (c d)"))
                x0_2, x1_2 = x2[:, 0:D], x2[:, D:2 * D]
                xb = prep.tile([32, 2 * D], F32, tag="xb")
                nc.scalar.dma_start(
                    out=xb[:],
                    in_=box1[128:160].rearrange("j c d -> j (c d)"))
                x0_b, x1_b = xb[:, 0:D], xb[:, D:2 * D]

                def chain(xs, p, nm):
                    """hh = fp16(255*[zmin | zmax]) per (x0, x1) in xs,
                    paired into one [p, len(xs)*2*D] ACT stream.

                    half0: v = sp - x0, zmin = sigmoid(-v) = exp(-ln(1+e^v))
                    half1: w = -(sp + x0), zmax = sigmoid(+sp+x0) =
                           exp(-ln(1+e^w));  sp = 0.1*ln(1+exp(-10|x1|))
                           + max(x1, 0).
                    """
                    nx = len(xs)
                    u1 = prep.tile([p, nx * D], F32, tag="u1",
                                   name=f"u1{nm}")
                    for k, (x0, x1) in enumerate(xs):
                        nc.vector.scalar_tensor_tensor(
                            out=u1[:, k * D:(k + 1) * D], in0=x1[:],
                            scalar=-1.0, in1=x1[:], op0=ALU.mult,
                            op1=ALU.max)
                    e1 = prep.tile([p, nx * D], F32, tag="e1",
                                   name=f"e1{nm}")
                    nc.scalar.activation(e1[:], u1[:], AF.Exp, scale=-10.0)
                    l1 = prep.tile([p, nx * D], F32, tag=f"l1{nm}",
                                   name=f"l1{nm}")
                    nc.scalar.activation(l1[:], e1[:], AF.Ln, bias=1.0)
                    vv = prep.tile([p, nx * 2 * D], F32, tag="vv",
                                   name=f"vv{nm}")
                    for k, (x0, x1) in enumerate(xs):
                        l1k = l1[:, k * D:(k + 1) * D]
                        q0 = prep.tile([p, D], F32, tag="q0",
                                       name=f"q0{nm}{k}")
                        nc.vector.scalar_tensor_tensor(
                            out=q0[:], in0=x1[:], scalar=0.0, in1=x0[:],
                            op0=ALU.max, op1=ALU.subtract)
                        nc.vector.scalar_tensor_tensor(
                            out=vv[:, (2 * k) * D:(2 * k + 1) * D],
                            in0=l1k, scalar=0.1, in1=q0[:],
                            op0=ALU.mult, op1=ALU.add)
                        q1 = prep.tile([p, D], F32, tag="q1",
                                       name=f"q1{nm}{k}")
                        nc.vector.scalar_tensor_tensor(
                            out=q1[:], in0=x1[:], scalar=0.0, in1=x0[:],
                            op0=ALU.max, op1=ALU.add)
                        nc.vector.scalar_tensor_tensor(
                            out=vv[:, (2 * k + 1) * D:(2 * k + 2) * D],
                            in0=l1k, scalar=-0.1, in1=q1[:],
                            op0=ALU.mult, op1=ALU.subtract)
                    ee = prep.tile([p, nx * 2 * D], F32, tag="u1",
                                   name=f"ee{nm}")
                    nc.scalar.activation(ee[:], vv[:], AF.Exp)
                    ll = prep.tile([p, nx * 2 * D], F32, tag="vv",
                                   name=f"ll{nm}")
                    nc.scalar.activation(ll[:], ee[:], AF.Ln, bias=1.0)
                    hh = prep.tile([p, nx * 2 * D], F16, tag="hh",
                                   name=f"hh{nm}")
                    nc.scalar.activation(hh[:], ll[:], AF.Exp, scale=-1.0,
                                         bias=ln255[0:p, :])
                    return [[hh[:, (2 * k) * D:(2 * k + 1) * D],
                             hh[:, (2 * k + 1) * D:(2 * k + 2) * D]]
                            for k in range(nx)]

                # box2 + box1a paired (both 128 partitions)
                h2, ha = chain([(x0_2, x1_2), (x0_a, x1_a)], 128, "2a")
                for t in range(2):
                    nc.vector.tensor_copy(
                        out=rep4k[t][:, 0:UNIT]
                        .rearrange("p (r d) -> p r d", d=D),
                        in_=h2[t][:, None, :]
                        .broadcast_to([SH, UNIT // D, D]))
                    nc.vector.tensor_copy(out=rep4k[t][:, UNIT:GCOL],
                                          in_=rep4k[t][:, 0:UNIT])
                for t in range(2):
                    ring = rings[t]
                    ring.dma_start(
                        out=zscr[t:t + 1, 0:128 * D]
                        .rearrange("t (r d) -> (t r) d", d=D),
                        in_=ha[t])
                    ring.dma_start(out=tab16[32 * t:32 * t + 1, 0:128 * D],
                                   in_=zscr[t:t + 1, 0:128 * D])
                    ring.dma_start(
                        out=tab16[64 + 32 * t:64 + 32 * t + 1, 0:128 * D],
                        in_=zscr[t:t + 1, 0:128 * D])

                # b-chunk tables
                (hb,) = chain([(x0_b, x1_b)], 32, "b")
                for t in range(2):
                    ring = rings[t]
                    ring.dma_start(
                        out=zscr[t:t + 1, 128 * D:FLAT]
                        .rearrange("t (r d) -> (t r) d", d=D),
                        in_=hb[t])
                    ring.dma_start(out=tab16[32 * t:32 * t + 1, 128 * D:FLAT],
                                   in_=zscr[t:t + 1, 128 * D:FLAT])
                    ring.dma_start(
                        out=tab16[64 + 32 * t:64 + 32 * t + 1, 128 * D:FLAT],
                        in_=zscr[t:t + 1, 128 * D:FLAT])

            # ---------------- main loop ----------------
            ops = [ALU.max, ALU.min]
            pats = [PAT0, PAT1]

            def mm(ps, t, off):
                for c in range(4):
                    row = 32 * t + 64 * (c % 2)
                    nc.tensor.matmul(
                        ps[:, c * 512:(c + 1) * 512],
                        lhsT=w16[row:row + 1, :],
                        rhs=tab16[row:row + 1,
                                  off + c * 512:off + c * 512 + 512],
                        start=True, stop=True, tile_position=(row, 0))

            def prod_a(t, g):
                zh = zhp.tile([128, GCOL], F16, tag="zh", name=f"zh{t}_{g}")
                for h in range(GW):
                    ps = psum.tile([128, UNIT], F32, tag="ps",
                                   name=f"ps{t}_{g}_{h}")
                    mm(ps, t, g * GCOL + h * UNIT)
                    nc.scalar.activation(zh[:, h * UNIT:(h + 1) * UNIT],
                                         ps[:], AF.Copy)
                return zh

            def prod_s(t, g):
                zb = zbcp.tile([128, GCOL], F16, tag="zbc",
                               name=f"zbs{t}_{g}")
                nc.gpsimd.dma_start(
                    out=zb[:],
                    in_=zscr[t:t + 1, g * GCOL:(g + 1) * GCOL]
                    .broadcast_to([128, GCOL]))
                return zb

            c_ctr = [0]

            def prod_c(t, g):
                zb = zbcp.tile([128, GCOL], F16, tag="zbc",
                               name=f"zbc{t}_{g}")
                c_ctr[0] += 1
                rings[c_ctr[0] % 2].dma_start(
                    out=zb[:],
                    in_=zscr[t:t + 1, g * GCOL:(g + 1) * GCOL]
                    .broadcast_to([128, GCOL]))
                return zb

            def prod_r(t, g):
                pss = []
                for h in range(GW):
                    ps = psum.tile([128, UNIT], F32, tag="ps",
                                   name=f"psr{t}_{g}_{h}")
                    mm(ps, t, g * GCOL + h * UNIT)
                    pss.append(ps)
                return pss

            st_ctr = [0]
            osb_pairs = {}
            st_ctr = [0]

            def cons_f16(t, g, zb):
                pair = g // 2
                if (t, pair) not in osb_pairs:
                    osb_pairs[(t, pair)] = osbp.tile(
                        [128, 2 * GCOL], F16, tag="osb",
                        name=f"osb{t}_{pair}")
                osb = osb_pairs[(t, pair)]
                sl = g % 2
                nc.vector.tensor_tensor(
                    out=osb[:, sl * GCOL:(sl + 1) * GCOL], in0=zb[:],
                    in1=rep4k[t][:], op=ops[t])
                if sl == 1:
                    j0 = pair * 2 * GJ
                    st_ctr[0] += 1
                    rings[st_ctr[0] % 2].dma_start(
                        out=outs[t][:, j0:j0 + 2 * GJ, :],
                        in_=osb_pairs.pop((t, pair))[:]
                        .rearrange("p (r d) -> p r d", d=D))

            def cons_r(t, g, pss):
                osb8 = osb8p.tile([128, GCOL], U8, tag="osb8",
                                  name=f"osb8{t}_{g}")
                for h in range(GW):
                    nc.vector.scalar_tensor_tensor(
                        out=osb8[:, h * UNIT:(h + 1) * UNIT], in0=pss[h][:],
                        scalar=0.0, in1=rep4k[t][:, 0:UNIT],
                        op0=ALU.bypass, op1=ops[t])
                j0 = (g - R_SPLIT[t]) * GJ
                nc.sync.dma_start(
                    out=outs8[t][:, j0:j0 + GJ, :],
                    in_=osb8[:].rearrange("p (r d) -> p r d", d=D))

            producers = {"a": prod_a, "c": prod_c, "s": prod_s,
                         "r": prod_r}


            jobs = []
            for k in range(NG):
                for t, ordl in ((0, ORD0), (1, ORD1)):
                    g = ordl[k]
                    jobs.append((t, g, pats[t][g]))

            staged = {}
            n = len(jobs)
            for i in range(n + LAG):
                if i < n:
                    t, g, p = jobs[i]
                    staged[i] = producers[p](t, g)
                k = i - LAG
                if k >= 0:
                    t, g, p = jobs[k]
                    if p == "r":
                        cons_r(t, g, staged.pop(k))
                    else:
                        cons_f16(t, g, staged.pop(k))
            # flush leftover half-filled fp16 pairs
            for (t, pair), osb in list(osb_pairs.items()):
                j0 = pair * 2 * GJ
                st_ctr[0] += 1
                rings[st_ctr[0] % 2].dma_start(
                    out=outs[t][:, j0:j0 + GJ, :],
                    in_=osb[:, 0:GCOL].rearrange("p (r d) -> p r d", d=D))
                del osb_pairs[(t, pair)]

    nc.compile()
    return nc


def _get_nc():
    if "nc" not in _CACHE:
        _CACHE["nc"] = _build()
    return _CACHE["nc"]


def make_in_maps(box1s, box2s):
    box1s = np.ascontiguousarray(np.asarray(box1s, dtype=np.float32))
    box2s = np.ascontiguousarray(np.asarray(box2s, dtype=np.float32))
    return [
        {
            "box1s": box1s,
            "box2s": np.ascontiguousarray(box2s[c * SH:(c + 1) * SH]),
        }
        for c in range(NCORES)
    ]


def kernel(box1s, box2s):
    nc = _get_nc()
    res = run_bass_kernel_spmd(nc, make_in_maps(box1s, box2s),
                               core_ids=list(range(NCORES)))
    inv = np.float32(1.0) / np.float32(255.0)

    def full(name, name8):
        parts = []
        for r in res.results:
            f = r[name].astype(np.float32)
            u = r[name8].astype(np.float32)
            parts.append(np.concatenate([f, u], axis=1).reshape(SH * N1, D))
        return np.concatenate(parts, axis=0) * inv

    return full("out_min", "out_min8"), full("out_max", "out_max8")
